# revision 17
# baseline (speedup 1.0000x reference)
"""Trainium2 Bass kernel: Llama-style attention prefill (B=2, S=2048, D=4096,
32 Q heads / 8 KV heads, head_dim 128, RoPE, additive mask), tensor-parallel
over heads across 8 NeuronCores.

Sharding (per core c):
  - Q heads 4c..4c+3 (wq columns c*512:(c+1)*512), KV head c (wk/wv columns
    c*128:(c+1)*128), wo column-shard wo[:, c*512:(c+1)*512].
  - Each core computes QKV projections + RoPE + attention for its heads,
    AllGathers the (transposed) attention outputs over all 8 cores (one AG
    per 512-token chunk, overlapped with compute), then computes a
    512-column slice of the output projection.
  - Host concatenates the 8 column slices -> full output.

Device-side layouts (all "T" = feature-on-partitions):
  - xT [4096 dm, 4096 tok] (tok = b*2048 + s), host-provided, bf16.
  - wq/wk columns are de-interleaved per head on the host: new col order
    [re pairs 0..63 | im pairs 0..63], so RoPE acts on partition halves.
    Scores are invariant (same permutation on Q and K); V/wo untouched.
  - Scores computed transposed: ST[k, t] = K @ Q^T; softmax over the
    partition axis k: exp (no max subtraction; |scores|*scale stays small
    so f32 exp is safe), then PV matmul with a ones-column appended to V
    producing both O[t, d] and the denominator L[t].
  - O^T staging for the AllGather is done ON-CHIP: each normalized o_n
    [128 t, 128 d] tile is transposed by the PE (is_transpose matmul with
    a bf16 identity -> bf16 PSUM tile), collected per head as [128 d,
    512 t], and DMA'd PSUM -> ag_in DRAM directly. No DRAM roundtrip and
    no xbar-mode DMA transposes (which would serialize against all
    previously-emitted collectives under the conservative xbar rule).
  - Scores/exp exploit causality at 128-row granularity: for the
    diagonal 512-block, key sub-block j only computes query columns
    >= j*128 (the PV accumulation skips the complementary tiles).
  - Emission: per qb, project the two 512-token blocks then run both
    chunks' attention + AllGather; final phase runs the qb=3 attention,
    triggers AG3, then all eight out-projection chunks (groups 0-2's
    AGs completed long ago; AG3 flies under the first six chunks).
    AG staging DMAs and PSUM->SBUF copies ride the otherwise-idle
    gpsimd queue so they are never stuck behind og/out traffic and
    each AG trigger fires as soon as its inputs land.
"""
import numpy as np
import ml_dtypes

from concourse import bass, bacc, tile, mybir, bass_utils
from concourse.masks import make_identity

F32 = mybir.dt.float32
BF16 = mybir.dt.bfloat16
Alu = mybir.AluOpType
Act = mybir.ActivationFunctionType

N_CORES = 8
B, S, D = 2, 2048, 4096
TOK = B * S                      # 4096 flattened tokens
HD = 128                         # head dim
HQ = 4                           # q heads per core
QW = HQ * HD                     # 512, per-core q width
SCALE = 1.0 / float(np.sqrt(HD))
NEG = -1e9

_BUILD_CACHE = {}


def _build(causal: bool):
    nc = bacc.Bacc("TRN2", target_bir_lowering=False, debug=False,
                   num_devices=N_CORES)
    # all inputs are pre-tiled on the host so every DMA is one contiguous
    # segment per partition (cheap descriptor generation)
    xT_d = nc.dram_tensor("xT", [8, 8, 128, 2048], BF16, kind="ExternalInput")
    wq_d = nc.dram_tensor("wq", [128, 32, QW], BF16, kind="ExternalInput")
    wk_d = nc.dram_tensor("wk", [128, 32, HD], BF16, kind="ExternalInput")
    wv_d = nc.dram_tensor("wv", [128, 32, HD], BF16, kind="ExternalInput")
    wo_d = nc.dram_tensor("wo", [128, 32, QW], BF16, kind="ExternalInput")
    cosT_d = nc.dram_tensor("cosT", [64, S], BF16, kind="ExternalInput")
    sinT_d = nc.dram_tensor("sinT", [64, S], BF16, kind="ExternalInput")
    if causal:
        # diagonal 512x512 blocks of mask^T, tiled [qb, j, 128, 512]
        maskd_d = nc.dram_tensor("maskd", [128, 4, 4, 512], BF16,
                                 kind="ExternalInput")
    else:
        maskT_d = nc.dram_tensor("maskT", [S, S], BF16, kind="ExternalInput")
    out_d = nc.dram_tensor("out", [TOK, QW], F32, kind="ExternalOutput")

    with tile.TileContext(nc) as tc:
        with (
            tc.tile_pool(name="res", bufs=1) as res,            # residents
            tc.tile_pool(name="qtp", bufs=4 if causal else 8) as qtp,
            tc.tile_pool(name="ph2", bufs=2) as ph2,
            tc.tile_pool(name="epool", bufs=18) as epool,
            tc.tile_pool(name="mpool", bufs=9) as mpool,
            tc.tile_pool(name="onp", bufs=4) as onp,
            tc.tile_pool(name="agd", bufs=8, space="DRAM") as agd,
            tc.tile_pool(name="psum", bufs=2, space="PSUM") as psum,
        ):
            # ---- resident loads (spread across queues) ----
            wk_sb = res.tile([128, 32, HD], BF16, name="wk_sb")
            # tiny first chunk so the very first matmul's LDWEIGHTS only
            # waits on a 32KB transfer (plus cold DMA-queue ramp)
            nc.gpsimd.dma_start(wk_sb[:, 0:1], wk_d[:, 0:1])
            nc.gpsimd.dma_start(wk_sb[:, 1:8], wk_d[:, 1:8])
            nc.gpsimd.dma_start(wk_sb[:, 8:32], wk_d[:, 8:32])
            wv_sb = res.tile([128, 32, HD], BF16, name="wv_sb")
            nc.gpsimd.dma_start(wv_sb[:], wv_d[:])
            cos_sb = res.tile([64, S], BF16, name="cos_sb")
            nc.gpsimd.dma_start(cos_sb[:], cosT_d[:])
            sin_sb = res.tile([64, S], BF16, name="sin_sb")
            nc.gpsimd.dma_start(sin_sb[:], sinT_d[:])
            if causal:
                mask_sb = res.tile([128, 4, 4, 512], BF16, name="mask_sb")
                nc.gpsimd.dma_start(mask_sb[:], maskd_d[:])
            ident = res.tile([128, 128], BF16, name="ident")
            make_identity(nc, ident[:])

            # per-token-block K/V residents (block granularity keeps the
            # projection->attention dependency tracking per-block)
            kt_t = [res.tile([128, 512], BF16, name=f"kt{tb}")
                    for tb in range(8)]
            v_t = [res.tile([128, 4, 130], BF16, name=f"v{tb}")
                   for tb in range(8)]
            for tb in range(8):
                nc.vector.memset(v_t[tb][:, :, 128:129], 1.0)

            # ---------------- projections + RoPE for one 512-token block --
            def rope_store(ps, out_re, out_im, cos_sl, sin_sl, rp):
                t1 = rp.tile([64, 512], F32, name="t1", tag="t1", bufs=1)
                t2 = rp.tile([64, 512], F32, name="t2", tag="t2", bufs=1)
                nc.vector.tensor_mul(t1[:], ps[0:64, :], cos_sl)
                nc.vector.tensor_mul(t2[:], ps[64:128, :], sin_sl)
                nc.vector.tensor_sub(out_re, t1[:], t2[:])
                nc.vector.tensor_mul(t1[:], ps[0:64, :], sin_sl)
                nc.vector.tensor_mul(t2[:], ps[64:128, :], cos_sl)
                nc.vector.tensor_add(out_im, t1[:], t2[:])

            def proj_block(tb, wq_sb, xtp, ph1):
                xts = []
                for g in range(8):
                    xt = xtp.tile([128, 2048], BF16, name="xt", tag="xt")
                    nc.sync.dma_start(xt[:], xT_d[tb, g])
                    xts.append(xt)

                def xsl(ic):
                    return xts[ic // 4][:, (ic % 4) * 512:(ic % 4 + 1) * 512]

                pos = (tb % 4) * 512
                cos_sl = cos_sb[:, pos:pos + 512]
                sin_sl = sin_sb[:, pos:pos + 512]

                ps_k = psum.tile([128, 512], F32, name="ps_k", tag="ps")
                for ic in range(32):
                    nc.tensor.matmul(ps_k[:], wk_sb[:, ic, :], xsl(ic),
                                     start=(ic == 0), stop=(ic == 31))
                rope_store(ps_k, kt_t[tb][0:64, :], kt_t[tb][64:128, :],
                           cos_sl, sin_sl, ph1)

                # V in natural [token, d] layout, computed directly:
                # lhsT = xT chunk [i, t(128)], rhs = wv chunk [i, d]
                for j in range(4):
                    ps_v = psum.tile([128, 128], F32, name="ps_v", tag="ps")
                    for ic in range(32):
                        nc.tensor.matmul(
                            ps_v[:],
                            xsl(ic)[:, j * 128:(j + 1) * 128],
                            wv_sb[:, ic, :],
                            start=(ic == 0), stop=(ic == 31))
                    nc.vector.tensor_copy(v_t[tb][:, j, 0:128], ps_v[:])

                qt = qtp.tile([128, HQ, 512], BF16, name="qt", tag="qt")
                for dq in range(HQ):
                    ps_q = psum.tile([128, 512], F32, name="ps_q", tag="ps")
                    for ic in range(32):
                        nc.tensor.matmul(
                            ps_q[:],
                            wq_sb[:, ic, dq * HD:(dq + 1) * HD],
                            xsl(ic),
                            start=(ic == 0), stop=(ic == 31))
                    rope_store(ps_q, qt[0:64, dq, :], qt[64:128, dq, :],
                               cos_sl, sin_sl, ph1)
                return qt

            # three AllGathers (A: qb0-1 chunks, B: qb2, C: qb3), sized so
            # AG-A/B trigger mid-kernel and AG-C hides under the out-
            # projections of A+B
            GRP_SLOTS = [2, 2, 2, 2]
            ag_in_g = [agd.tile([512, 512 * n], BF16, name=f"ag_in{g}")
                       for g, n in enumerate(GRP_SLOTS)]
            ag_out_g = [agd.tile([D, 512 * n], BF16, name=f"ag_out{g}",
                                 addr_space="Shared")
                        for g, n in enumerate(GRP_SLOTS)]

            def chunk_group(qb, b):
                # (group, slot) for chunk (qb, b)
                return qb, b

            # ---------------- attention + AG staging for one chunk --------
            def attn_chunk(qb, b, qt, mtiles, kcs, qoff):
                # qoff[kc]: first valid query column for block kc (causal
                # diagonal sub-blocks only attend q >= k, so the scores /
                # exp / PV for q < qoff[kc] are skipped entirely)
                g, slot = chunk_group(qb, b)
                for h in range(HQ):
                    e_ps = {}
                    for kc in kcs:
                        qo = qoff.get(kc, 0)
                        st_ps = psum.tile([128, 512], F32, name="st_ps",
                                          tag="st")
                        tbk = b * 4 + kc // 4
                        kof = (kc % 4) * 128
                        nc.tensor.matmul(
                            st_ps[:, qo:],
                            kt_t[tbk][:, kof:kof + 128],
                            qt[:, h, qo:],
                            start=True, stop=True)
                        e_t = epool.tile([128, 512], BF16, name="e_t",
                                         tag="e")
                        if mtiles.get(kc) is not None:
                            pre = ph2.tile([128, 512], F32, name="pre",
                                           tag="pre")
                            nc.vector.scalar_tensor_tensor(
                                pre[:, qo:], st_ps[:, qo:], SCALE,
                                mtiles[kc][:, qo:], Alu.mult, Alu.add)
                            nc.scalar.activation(e_t[:, qo:], pre[:, qo:],
                                                 Act.Exp)
                        else:
                            nc.scalar.activation(e_t[:, qo:], st_ps[:, qo:],
                                                 Act.Exp, scale=SCALE)
                        e_ps[kc] = e_t
                    # per-head transposed output collector [128 d, 512 t],
                    # bf16 PSUM (PE transpose writes lhsT dtype)
                    tr = psum.tile([128, 512], BF16, name="tr", tag="tr")
                    for ts in range(4):
                        tcs = [kc for kc in kcs
                               if qoff.get(kc, 0) <= ts * 128]
                        o_ps = psum.tile([128, 129], F32, name="o_ps", tag="o")
                        for i, kc in enumerate(tcs):
                            nc.tensor.matmul(
                                o_ps[:],
                                e_ps[kc][:, ts * 128:(ts + 1) * 128],
                                v_t[b * 4 + kc // 4][:, kc % 4, 0:129],
                                start=(i == 0),
                                stop=(i == len(tcs) - 1))
                        linv = onp.tile([128, 1], F32, name="linv", tag="linv")
                        nc.vector.reciprocal(linv[:], o_ps[:, 128:129])
                        o_n = onp.tile([128, 128], BF16, name="o_n", tag="o_n")
                        nc.vector.tensor_scalar(
                            o_n[:], o_ps[:, 0:128], linv[:], None, Alu.mult)
                        nc.tensor.transpose(
                            tr[:, ts * 128:(ts + 1) * 128], o_n[:], ident[:])
                    trs = onp.tile([128, 512], BF16, name="trs", tag="trs")
                    # copy on vector: the scalar engine is co-critical with
                    # the PE during attention (exp stream); gpsimd cannot
                    # read PSUM
                    nc.vector.tensor_copy(trs[:], tr[:])
                    # staging DMAs ride the gpsimd queue: it holds only the
                    # AG triggers, so staging is never queued behind og /
                    # out traffic and each trigger fires right after its
                    # inputs land
                    nc.gpsimd.dma_start(
                        ag_in_g[g][h * 128:(h + 1) * 128,
                                   slot * 512:(slot + 1) * 512],
                        trs[:])

            def ag_group(g):
                nc.gpsimd.collective_compute(
                    "AllGather", Alu.bypass,
                    replica_groups=[list(range(N_CORES))],
                    ins=[ag_in_g[g][:]], outs=[ag_out_g[g][:]])

            # ---------------- emission ------------------------------------
            def make_mtiles(qb):
                if causal:
                    kcs = list(range(4 * qb + 4))
                    mtiles = {4 * qb + j: mask_sb[:, qb, j, :]
                              for j in range(4)}
                    qoff = {4 * qb + j: j * 128 for j in range(4)}
                    return mtiles, kcs, qoff
                else:
                    kcs = list(range(16))
                    mtiles = {}
                    for kc in range(0, 16, 2):
                        mt = mpool.tile([128, 2, 512], BF16, name="mt",
                                        tag="mt")
                        nc.sync.dma_start(
                            mt[:],
                            maskT_d[kc * 128:(kc + 2) * 128,
                                    qb * 512:(qb + 1) * 512]
                            .rearrange("(two p) t -> p two t", p=128))
                        mtiles[kc] = mt[:, 0, :]
                        mtiles[kc + 1] = mt[:, 1, :]
                return mtiles, kcs, {}

            with (
                tc.tile_pool(name="wqp", bufs=1) as wqp,
                tc.tile_pool(name="xtp", bufs=16 if causal else 8) as xtp,
                tc.tile_pool(name="ph1", bufs=2) as ph1,
            ):
                wq_sb = wqp.tile([128, 32, QW], BF16, name="wq_sb")
                nc.scalar.dma_start(wq_sb[:], wq_d[:])
                # tiny dummy AllGather emitted after the weight loads (the
                # collective trigger blocks the gpsimd queue): aligns the
                # cores' start skew while the first projection blocks keep
                # the PE busy, so AG0 isn't delayed by launch offset
                dummy_in = agd.tile([64, 4], BF16, name="dummy_in")
                nc.gpsimd.dma_start(dummy_in[:], cosT_d[0:64, 0:4])
                dummy_out = agd.tile([512, 4], BF16, name="dummy_out",
                                     addr_space="Shared")
                nc.gpsimd.collective_compute(
                    "AllGather", Alu.bypass,
                    replica_groups=[list(range(N_CORES))],
                    ins=[dummy_in[:]], outs=[dummy_out[:]])
                if causal:
                    # causal: chunk (qb, b) needs only blocks <= qb, so
                    # projections and attention interleave per qb
                    for qb in range(4):
                        qts = [proj_block(b * 4 + qb, wq_sb, xtp, ph1)
                               for b in range(B)]
                        mtiles, kcs, qoff = make_mtiles(qb)
                        if qb < 3:
                            for b in range(B):
                                attn_chunk(qb, b, qts[b], mtiles, kcs, qoff)
                            ag_group(qb)
                        else:
                            qts3, mtiles3, kcs3, qoff3 = qts, mtiles, kcs, \
                                qoff
                else:
                    # general mask: every chunk may attend to every block,
                    # so all projections must complete first
                    allq = [[None, None] for _ in range(4)]
                    for qb in range(4):
                        for b in range(B):
                            allq[qb][b] = proj_block(b * 4 + qb, wq_sb, xtp,
                                                     ph1)
                    for qb in range(3):
                        mtiles, kcs, qoff = make_mtiles(qb)
                        for b in range(B):
                            attn_chunk(qb, b, allq[qb][b], mtiles, kcs, qoff)
                        ag_group(qb)
                    qts3 = allq[3]
                    mtiles3, kcs3, qoff3 = make_mtiles(3)

            # final phase: wo reuses wq's SBUF lifetime. Order: qb=3
            # attention chunks (staging lands in ag_in[3]), then the six
            # out-projection chunks of groups 0-2 (their AGs completed long
            # before), with AG3 - whose gpsimd trigger has nothing queued
            # ahead of it but the wo load - flying underneath, then the two
            # group-3 out-projections.
            with (
                tc.tile_pool(name="wop", bufs=1) as wop,
                tc.tile_pool(name="ogp", bufs=2) as ogp,
            ):
                wo_sb = wop.tile([128, 32, QW], BF16, name="wo_sb")
                nc.gpsimd.dma_start(wo_sb[:], wo_d[:])

                def out_proj(ag_out, slot, t0):
                    for half in range(2):
                        og_sb = ogp.tile([128, 32, 256], BF16, name="og_sb",
                                         tag="og")
                        nc.sync.dma_start(
                            og_sb[:],
                            ag_out[:, slot * 512 + half * 256:
                                   slot * 512 + (half + 1) * 256]
                            .rearrange("(hc p) t -> p hc t", p=128))
                        for tci in range(2):
                            tc2 = half * 2 + tci
                            op_ps = psum.tile([128, 512], F32, name="op_ps",
                                              tag="ps")
                            for hc in range(32):
                                nc.tensor.matmul(
                                    op_ps[:],
                                    og_sb[:, hc, tci * 128:(tci + 1) * 128],
                                    wo_sb[:, hc, :],
                                    start=(hc == 0), stop=(hc == 31))
                            oo = ph2.tile([128, 512], F32, name="oo",
                                          tag="oo")
                            nc.vector.tensor_copy(oo[:], op_ps[:])
                            nc.scalar.dma_start(
                                out_d[t0 + tc2 * 128: t0 + (tc2 + 1) * 128,
                                      :],
                                oo[:])

                for b3 in range(B):
                    attn_chunk(3, b3, qts3[b3], mtiles3, kcs3, qoff3)
                ag_group(3)
                for qb in range(4):
                    for b in range(B):
                        g, slot = chunk_group(qb, b)
                        out_proj(ag_out_g[g], slot, b * S + qb * 512)

    nc.compile()
    return nc


def _get_nc(causal: bool):
    if causal not in _BUILD_CACHE:
        _BUILD_CACHE[causal] = _build(causal)
    return _BUILD_CACHE[causal]


_DEINT = np.concatenate([np.arange(0, HD, 2), np.arange(1, HD, 2)])


def _deinterleave(w):
    """Permute per-head columns [0,2,..,126,1,3,..,127] (re block | im block)."""
    out = w.copy()
    nh = w.shape[1] // HD
    for h in range(nh):
        out[:, h * HD:(h + 1) * HD] = w[:, h * HD + _DEINT]
    return out


def make_in_maps(x, wq, wk, wv, wo, freqs_cos, freqs_sin, mask, causal):
    bf = ml_dtypes.bfloat16

    def tile_w(w):
        # [4096, W] -> [128, 32, W]: out[p, ic, d] = w[ic*128+p, d]
        return np.ascontiguousarray(
            w.reshape(32, 128, -1).transpose(1, 0, 2)).astype(bf)

    xT = np.asarray(x, np.float32).reshape(TOK, D).T        # [dm, tok]
    # [tb, g, p, ic_in*512+t] = xT[g*512+ic_in*128+p, tb*512+t]
    xt_host = np.ascontiguousarray(
        xT.reshape(8, 4, 128, 8, 512).transpose(3, 0, 2, 1, 4)
        .reshape(8, 8, 128, 2048)).astype(bf)
    cosT = np.ascontiguousarray(np.asarray(freqs_cos, np.float32).T).astype(bf)
    sinT = np.ascontiguousarray(np.asarray(freqs_sin, np.float32).T).astype(bf)
    mask = np.asarray(mask, np.float32)

    in_maps = []
    for c in range(N_CORES):
        m = {
            "xT": xt_host,
            "wq": tile_w(_deinterleave(
                np.asarray(wq[:, c * QW:(c + 1) * QW], np.float32))),
            "wk": tile_w(_deinterleave(
                np.asarray(wk[:, c * HD:(c + 1) * HD], np.float32))),
            "wv": tile_w(np.asarray(wv[:, c * HD:(c + 1) * HD], np.float32)),
            "wo": tile_w(np.asarray(wo[:, c * QW:(c + 1) * QW], np.float32)),
            "cosT": cosT,
            "sinT": sinT,
        }
        if causal:
            md = np.empty((4, 4, 128, 512), np.float32)
            for qb in range(4):
                blk = mask[qb * 512:(qb + 1) * 512,
                           qb * 512:(qb + 1) * 512]          # [q, k]
                md[qb] = blk.T.reshape(4, 128, 512)          # [j, k128, q512]
            m["maskd"] = np.ascontiguousarray(
                md.transpose(2, 0, 1, 3)).astype(bf)         # [p, qb, j, t]
        else:
            m["maskT"] = np.ascontiguousarray(mask.T).astype(bf)
        in_maps.append(m)
    return in_maps


def _is_causal(mask):
    mask = np.asarray(mask, np.float32)
    expect = np.where(np.tril(np.ones((S, S), bool)), 0.0, NEG).astype(np.float32)
    return np.array_equal(mask, expect)


def kernel(x, wq, wk, wv, wo, cache_k, cache_v, freqs_cos, freqs_sin, mask,
           start_pos):
    assert int(start_pos) == 0, "kernel hardcodes start_pos=0 prefill"
    assert tuple(np.shape(x)) == (B, S, D)
    causal = _is_causal(mask)
    nc = _get_nc(causal)
    in_maps = make_in_maps(x, wq, wk, wv, wo, freqs_cos, freqs_sin, mask,
                           causal)
    res = bass_utils.run_bass_kernel_spmd(
        nc, in_maps, core_ids=list(range(N_CORES)))
    out = np.empty((TOK, D), np.float32)
    for c in range(N_CORES):
        out[:, c * QW:(c + 1) * QW] = res.results[c]["out"]
    return out.reshape(B, S, D)


# revision 19
# speedup vs baseline: 1.0231x; 1.0231x over previous
"""Trainium2 Bass kernel: Llama-style attention prefill (B=2, S=2048, D=4096,
32 Q heads / 8 KV heads, head_dim 128, RoPE, additive mask), tensor-parallel
over heads across 8 NeuronCores.

Sharding (per core c):
  - Q heads 4c..4c+3 (wq columns c*512:(c+1)*512), KV head c (wk/wv columns
    c*128:(c+1)*128), wo column-shard wo[:, c*512:(c+1)*512].
  - Each core computes QKV projections + RoPE + attention for its heads,
    AllGathers the (transposed) attention outputs over all 8 cores (one AG
    per 512-token chunk, overlapped with compute), then computes a
    512-column slice of the output projection.
  - Host concatenates the 8 column slices -> full output.

Device-side layouts (all "T" = feature-on-partitions):
  - xT [4096 dm, 4096 tok] (tok = b*2048 + s), host-provided, bf16.
  - wq/wk columns are de-interleaved per head on the host: new col order
    [re pairs 0..63 | im pairs 0..63], so RoPE acts on partition halves.
    Scores are invariant (same permutation on Q and K); V/wo untouched.
  - Scores computed transposed: ST[k, t] = K @ Q^T; softmax over the
    partition axis k: exp (no max subtraction; |scores|*scale stays small
    so f32 exp is safe), then PV matmul with a ones-column appended to V
    producing both O[t, d] and the denominator L[t].
  - O^T staging for the AllGather is done ON-CHIP: each normalized o_n
    [128 t, 128 d] tile is transposed by the PE (is_transpose matmul with
    a bf16 identity -> bf16 PSUM tile), collected per head as [128 d,
    512 t], and DMA'd PSUM -> ag_in DRAM directly. No DRAM roundtrip and
    no xbar-mode DMA transposes (which would serialize against all
    previously-emitted collectives under the conservative xbar rule).
  - Scores/exp exploit causality at 128-row granularity: for the
    diagonal 512-block, key sub-block j only computes query columns
    >= j*128 (the PV accumulation skips the complementary tiles).
  - Emission: per qb, project the two 512-token blocks then run both
    chunks' attention + AllGather; final phase runs the qb=3 attention,
    triggers AG3, then all eight out-projection chunks (groups 0-2's
    AGs completed long ago; AG3 flies under the first six chunks).
    AG staging DMAs and PSUM->SBUF copies ride the otherwise-idle
    gpsimd queue so they are never stuck behind og/out traffic and
    each AG trigger fires as soon as its inputs land.
"""
import numpy as np
import ml_dtypes

from concourse import bass, bacc, tile, mybir, bass_utils
from concourse.masks import make_identity

F32 = mybir.dt.float32
BF16 = mybir.dt.bfloat16
Alu = mybir.AluOpType
Act = mybir.ActivationFunctionType

N_CORES = 8
B, S, D = 2, 2048, 4096
TOK = B * S                      # 4096 flattened tokens
HD = 128                         # head dim
HQ = 4                           # q heads per core
QW = HQ * HD                     # 512, per-core q width
SCALE = 1.0 / float(np.sqrt(HD))
NEG = -1e9

_BUILD_CACHE = {}


def _build(causal: bool):
    nc = bacc.Bacc("TRN2", target_bir_lowering=False, debug=False,
                   num_devices=N_CORES)
    # all inputs are pre-tiled on the host so every DMA is one contiguous
    # segment per partition (cheap descriptor generation)
    xT_d = nc.dram_tensor("xT", [8, 8, 128, 2048], BF16, kind="ExternalInput")
    wq_d = nc.dram_tensor("wq", [128, 32, QW], BF16, kind="ExternalInput")
    wk_d = nc.dram_tensor("wk", [128, 32, HD], BF16, kind="ExternalInput")
    wv_d = nc.dram_tensor("wv", [128, 32, HD], BF16, kind="ExternalInput")
    wo_d = nc.dram_tensor("wo", [128, 32, QW], BF16, kind="ExternalInput")
    cosT_d = nc.dram_tensor("cosT", [64, S], BF16, kind="ExternalInput")
    sinT_d = nc.dram_tensor("sinT", [64, S], BF16, kind="ExternalInput")
    if causal:
        # diagonal 512x512 blocks of mask^T, tiled [qb, j, 128, 512]
        maskd_d = nc.dram_tensor("maskd", [128, 4, 4, 512], BF16,
                                 kind="ExternalInput")
    else:
        maskT_d = nc.dram_tensor("maskT", [S, S], BF16, kind="ExternalInput")
    out_d = nc.dram_tensor("out", [TOK, QW], F32, kind="ExternalOutput")

    with tile.TileContext(nc) as tc:
        with (
            tc.tile_pool(name="res", bufs=1) as res,            # residents
            tc.tile_pool(name="qtp", bufs=4 if causal else 8) as qtp,
            tc.tile_pool(name="ph2", bufs=2) as ph2,
            tc.tile_pool(name="epool", bufs=18) as epool,
            tc.tile_pool(name="mpool", bufs=9) as mpool,
            tc.tile_pool(name="onp", bufs=4) as onp,
            tc.tile_pool(name="agd", bufs=8, space="DRAM") as agd,
            tc.tile_pool(name="psum", bufs=2, space="PSUM") as psum,
        ):
            # ---- resident loads (spread across queues) ----
            wk_sb = res.tile([128, 32, HD], BF16, name="wk_sb")
            # small first chunk so the very first matmul's LDWEIGHTS isn't
            # gated on the whole 1MB tile (plus cold DMA-queue ramp)
            nc.gpsimd.dma_start(wk_sb[:, 0:4], wk_d[:, 0:4])
            nc.gpsimd.dma_start(wk_sb[:, 4:32], wk_d[:, 4:32])
            wv_sb = res.tile([128, 32, HD], BF16, name="wv_sb")
            nc.gpsimd.dma_start(wv_sb[:], wv_d[:])
            cos_sb = res.tile([64, S], BF16, name="cos_sb")
            nc.gpsimd.dma_start(cos_sb[:], cosT_d[:])
            sin_sb = res.tile([64, S], BF16, name="sin_sb")
            nc.gpsimd.dma_start(sin_sb[:], sinT_d[:])
            if causal:
                mask_sb = res.tile([128, 4, 4, 512], BF16, name="mask_sb")
                nc.gpsimd.dma_start(mask_sb[:], maskd_d[:])
            ident = res.tile([128, 128], BF16, name="ident")
            make_identity(nc, ident[:])

            # per-token-block K/V residents (block granularity keeps the
            # projection->attention dependency tracking per-block)
            kt_t = [res.tile([128, 512], BF16, name=f"kt{tb}")
                    for tb in range(8)]
            v_t = [res.tile([128, 4, 130], BF16, name=f"v{tb}")
                   for tb in range(8)]
            for tb in range(8):
                nc.vector.memset(v_t[tb][:, :, 128:129], 1.0)

            # ---------------- projections + RoPE for one 512-token block --
            def rope_store(ps, out_re, out_im, cos_sl, sin_sl, rp):
                t1 = rp.tile([64, 512], F32, name="t1", tag="t1", bufs=1)
                t2 = rp.tile([64, 512], F32, name="t2", tag="t2", bufs=1)
                nc.vector.tensor_mul(t1[:], ps[0:64, :], cos_sl)
                nc.vector.tensor_mul(t2[:], ps[64:128, :], sin_sl)
                nc.vector.tensor_sub(out_re, t1[:], t2[:])
                nc.vector.tensor_mul(t1[:], ps[0:64, :], sin_sl)
                nc.vector.tensor_mul(t2[:], ps[64:128, :], cos_sl)
                nc.vector.tensor_add(out_im, t1[:], t2[:])

            def proj_block(tb, wq_sb, xtp, ph1):
                xts = []
                for g in range(8):
                    xt = xtp.tile([128, 2048], BF16, name="xt", tag="xt")
                    nc.sync.dma_start(xt[:], xT_d[tb, g])
                    xts.append(xt)

                def xsl(ic):
                    return xts[ic // 4][:, (ic % 4) * 512:(ic % 4 + 1) * 512]

                pos = (tb % 4) * 512
                cos_sl = cos_sb[:, pos:pos + 512]
                sin_sl = sin_sb[:, pos:pos + 512]

                ps_k = psum.tile([128, 512], F32, name="ps_k", tag="ps")
                for ic in range(32):
                    nc.tensor.matmul(ps_k[:], wk_sb[:, ic, :], xsl(ic),
                                     start=(ic == 0), stop=(ic == 31))
                rope_store(ps_k, kt_t[tb][0:64, :], kt_t[tb][64:128, :],
                           cos_sl, sin_sl, ph1)

                # V in natural [token, d] layout, computed directly:
                # lhsT = xT chunk [i, t(128)], rhs = wv chunk [i, d]
                for j in range(4):
                    ps_v = psum.tile([128, 128], F32, name="ps_v", tag="ps")
                    for ic in range(32):
                        nc.tensor.matmul(
                            ps_v[:],
                            xsl(ic)[:, j * 128:(j + 1) * 128],
                            wv_sb[:, ic, :],
                            start=(ic == 0), stop=(ic == 31))
                    nc.vector.tensor_copy(v_t[tb][:, j, 0:128], ps_v[:])

                qt = qtp.tile([128, HQ, 512], BF16, name="qt", tag="qt")
                for dq in range(HQ):
                    ps_q = psum.tile([128, 512], F32, name="ps_q", tag="ps")
                    for ic in range(32):
                        nc.tensor.matmul(
                            ps_q[:],
                            wq_sb[:, ic, dq * HD:(dq + 1) * HD],
                            xsl(ic),
                            start=(ic == 0), stop=(ic == 31))
                    rope_store(ps_q, qt[0:64, dq, :], qt[64:128, dq, :],
                               cos_sl, sin_sl, ph1)
                return qt

            # three AllGathers (A: qb0-1 chunks, B: qb2, C: qb3), sized so
            # AG-A/B trigger mid-kernel and AG-C hides under the out-
            # projections of A+B
            GRP_SLOTS = [2, 2, 2, 2]
            ag_in_g = [agd.tile([512, 512 * n], BF16, name=f"ag_in{g}")
                       for g, n in enumerate(GRP_SLOTS)]
            ag_out_g = [agd.tile([D, 512 * n], BF16, name=f"ag_out{g}",
                                 addr_space="Shared")
                        for g, n in enumerate(GRP_SLOTS)]

            def chunk_group(qb, b):
                # (group, slot) for chunk (qb, b)
                return qb, b

            # ---------------- attention + AG staging for one chunk --------
            def attn_chunk(qb, b, qt, mtiles, kcs, qoff):
                # qoff[kc]: first valid query column for block kc (causal
                # diagonal sub-blocks only attend q >= k, so the scores /
                # exp / PV for q < qoff[kc] are skipped entirely)
                g, slot = chunk_group(qb, b)
                for h in range(HQ):
                    e_ps = {}
                    for kc in kcs:
                        qo = qoff.get(kc, 0)
                        st_ps = psum.tile([128, 512], F32, name="st_ps",
                                          tag="st")
                        tbk = b * 4 + kc // 4
                        kof = (kc % 4) * 128
                        nc.tensor.matmul(
                            st_ps[:, qo:],
                            kt_t[tbk][:, kof:kof + 128],
                            qt[:, h, qo:],
                            start=True, stop=True)
                        e_t = epool.tile([128, 512], BF16, name="e_t",
                                         tag="e")
                        if mtiles.get(kc) is not None:
                            pre = ph2.tile([128, 512], F32, name="pre",
                                           tag="pre")
                            nc.vector.scalar_tensor_tensor(
                                pre[:, qo:], st_ps[:, qo:], SCALE,
                                mtiles[kc][:, qo:], Alu.mult, Alu.add)
                            nc.scalar.activation(e_t[:, qo:], pre[:, qo:],
                                                 Act.Exp)
                        else:
                            nc.scalar.activation(e_t[:, qo:], st_ps[:, qo:],
                                                 Act.Exp, scale=SCALE)
                        e_ps[kc] = e_t
                    # per-head transposed output collector [128 d, 512 t],
                    # bf16 PSUM (PE transpose writes lhsT dtype)
                    tr = psum.tile([128, 512], BF16, name="tr", tag="tr")
                    for ts in range(4):
                        tcs = [kc for kc in kcs
                               if qoff.get(kc, 0) <= ts * 128]
                        o_ps = psum.tile([128, 129], F32, name="o_ps", tag="o")
                        for i, kc in enumerate(tcs):
                            nc.tensor.matmul(
                                o_ps[:],
                                e_ps[kc][:, ts * 128:(ts + 1) * 128],
                                v_t[b * 4 + kc // 4][:, kc % 4, 0:129],
                                start=(i == 0),
                                stop=(i == len(tcs) - 1))
                        linv = onp.tile([128, 1], F32, name="linv", tag="linv")
                        nc.vector.reciprocal(linv[:], o_ps[:, 128:129])
                        o_n = onp.tile([128, 128], BF16, name="o_n", tag="o_n")
                        nc.vector.tensor_scalar(
                            o_n[:], o_ps[:, 0:128], linv[:], None, Alu.mult)
                        nc.tensor.transpose(
                            tr[:, ts * 128:(ts + 1) * 128], o_n[:], ident[:])
                    trs = onp.tile([128, 512], BF16, name="trs", tag="trs")
                    # copy on vector: the scalar engine is co-critical with
                    # the PE during attention (exp stream); gpsimd cannot
                    # read PSUM
                    nc.vector.tensor_copy(trs[:], tr[:])
                    # staging DMAs ride the gpsimd queue: it holds only the
                    # AG triggers, so staging is never queued behind og /
                    # out traffic and each trigger fires right after its
                    # inputs land
                    nc.gpsimd.dma_start(
                        ag_in_g[g][h * 128:(h + 1) * 128,
                                   slot * 512:(slot + 1) * 512],
                        trs[:])

            def ag_group(g):
                nc.gpsimd.collective_compute(
                    "AllGather", Alu.bypass,
                    replica_groups=[list(range(N_CORES))],
                    ins=[ag_in_g[g][:]], outs=[ag_out_g[g][:]])

            # ---------------- emission ------------------------------------
            def make_mtiles(qb):
                if causal:
                    kcs = list(range(4 * qb + 4))
                    mtiles = {4 * qb + j: mask_sb[:, qb, j, :]
                              for j in range(4)}
                    qoff = {4 * qb + j: j * 128 for j in range(4)}
                    return mtiles, kcs, qoff
                else:
                    kcs = list(range(16))
                    mtiles = {}
                    for kc in range(0, 16, 2):
                        mt = mpool.tile([128, 2, 512], BF16, name="mt",
                                        tag="mt")
                        nc.sync.dma_start(
                            mt[:],
                            maskT_d[kc * 128:(kc + 2) * 128,
                                    qb * 512:(qb + 1) * 512]
                            .rearrange("(two p) t -> p two t", p=128))
                        mtiles[kc] = mt[:, 0, :]
                        mtiles[kc + 1] = mt[:, 1, :]
                return mtiles, kcs, {}

            with (
                tc.tile_pool(name="wqp", bufs=1) as wqp,
                tc.tile_pool(name="xtp", bufs=16 if causal else 8) as xtp,
                tc.tile_pool(name="ph1", bufs=2) as ph1,
            ):
                wq_sb = wqp.tile([128, 32, QW], BF16, name="wq_sb")
                nc.scalar.dma_start(wq_sb[:], wq_d[:])
                # tiny dummy AllGather emitted after the weight loads (the
                # collective trigger blocks the gpsimd queue): aligns the
                # cores' start skew while the first projection blocks keep
                # the PE busy, so AG0 isn't delayed by launch offset
                dummy_in = agd.tile([64, 4], BF16, name="dummy_in")
                nc.gpsimd.dma_start(dummy_in[:], cosT_d[0:64, 0:4])
                dummy_out = agd.tile([512, 4], BF16, name="dummy_out",
                                     addr_space="Shared")
                nc.gpsimd.collective_compute(
                    "AllGather", Alu.bypass,
                    replica_groups=[list(range(N_CORES))],
                    ins=[dummy_in[:]], outs=[dummy_out[:]])
                if causal:
                    # causal: chunk (qb, b) needs only blocks <= qb, so
                    # projections and attention interleave per qb
                    for qb in range(4):
                        qts = [proj_block(b * 4 + qb, wq_sb, xtp, ph1)
                               for b in range(B)]
                        mtiles, kcs, qoff = make_mtiles(qb)
                        if qb < 3:
                            for b in range(B):
                                attn_chunk(qb, b, qts[b], mtiles, kcs, qoff)
                            ag_group(qb)
                        else:
                            qts3, mtiles3, kcs3, qoff3 = qts, mtiles, kcs, \
                                qoff
                else:
                    # general mask: every chunk may attend to every block,
                    # so all projections must complete first
                    allq = [[None, None] for _ in range(4)]
                    for qb in range(4):
                        for b in range(B):
                            allq[qb][b] = proj_block(b * 4 + qb, wq_sb, xtp,
                                                     ph1)
                    for qb in range(3):
                        mtiles, kcs, qoff = make_mtiles(qb)
                        for b in range(B):
                            attn_chunk(qb, b, allq[qb][b], mtiles, kcs, qoff)
                        ag_group(qb)
                    qts3 = allq[3]
                    mtiles3, kcs3, qoff3 = make_mtiles(3)

            # final phase: wo reuses wq's SBUF lifetime. Order: qb=3
            # attention chunks (staging lands in ag_in[3]), then the six
            # out-projection chunks of groups 0-2 (their AGs completed long
            # before), with AG3 - whose gpsimd trigger has nothing queued
            # ahead of it but the wo load - flying underneath, then the two
            # group-3 out-projections.
            with (
                tc.tile_pool(name="wop", bufs=1) as wop,
                tc.tile_pool(name="ogp", bufs=4) as ogp,
            ):
                wo_sb = wop.tile([128, 32, QW], BF16, name="wo_sb")
                nc.gpsimd.dma_start(wo_sb[:], wo_d[:])

                def og_load(ag_out, slot, tc2):
                    # quarter-chunk og tile; 4-deep ring gives ~3 loads of
                    # prefetch so the attn3 -> out-proj boundary and og/AG
                    # HBM contention never stall the PE
                    og_sb = ogp.tile([128, 32, 128], BF16, name="og_sb",
                                     tag="og")
                    nc.sync.dma_start(
                        og_sb[:],
                        ag_out[:, slot * 512 + tc2 * 128:
                               slot * 512 + (tc2 + 1) * 128]
                        .rearrange("(hc p) t -> p hc t", p=128))
                    return og_sb

                def out_proj_q(og_sb, t0, tc2):
                    op_ps = psum.tile([128, 512], F32, name="op_ps",
                                      tag="ps")
                    for hc in range(32):
                        nc.tensor.matmul(
                            op_ps[:],
                            og_sb[:, hc, :],
                            wo_sb[:, hc, :],
                            start=(hc == 0), stop=(hc == 31))
                    oo = ph2.tile([128, 512], F32, name="oo", tag="oo")
                    nc.vector.tensor_copy(oo[:], op_ps[:])
                    nc.scalar.dma_start(
                        out_d[t0 + tc2 * 128: t0 + (tc2 + 1) * 128, :],
                        oo[:])

                for b3 in range(B):
                    attn_chunk(3, b3, qts3[b3], mtiles3, kcs3, qoff3)
                # out-projections for groups 0-2 are emitted BEFORE the
                # final collective so their og loads can never order
                # behind it; AG3's trigger sits on the clean gpsimd queue
                # right after the attn3 staging DMAs and flies under them
                quarters = []
                for qb in range(4):
                    for b in range(B):
                        g, slot = chunk_group(qb, b)
                        for tc2 in range(4):
                            quarters.append((g, slot, b * S + qb * 512, tc2))
                for g, slot, t0, tc2 in quarters[:24]:
                    out_proj_q(og_load(ag_out_g[g], slot, tc2), t0, tc2)
                ag_group(3)
                for g, slot, t0, tc2 in quarters[24:]:
                    out_proj_q(og_load(ag_out_g[g], slot, tc2), t0, tc2)

    nc.compile()
    return nc


def _get_nc(causal: bool):
    if causal not in _BUILD_CACHE:
        _BUILD_CACHE[causal] = _build(causal)
    return _BUILD_CACHE[causal]


_DEINT = np.concatenate([np.arange(0, HD, 2), np.arange(1, HD, 2)])


def _deinterleave(w):
    """Permute per-head columns [0,2,..,126,1,3,..,127] (re block | im block)."""
    out = w.copy()
    nh = w.shape[1] // HD
    for h in range(nh):
        out[:, h * HD:(h + 1) * HD] = w[:, h * HD + _DEINT]
    return out


def make_in_maps(x, wq, wk, wv, wo, freqs_cos, freqs_sin, mask, causal):
    bf = ml_dtypes.bfloat16

    def tile_w(w):
        # [4096, W] -> [128, 32, W]: out[p, ic, d] = w[ic*128+p, d]
        return np.ascontiguousarray(
            w.reshape(32, 128, -1).transpose(1, 0, 2)).astype(bf)

    xT = np.asarray(x, np.float32).reshape(TOK, D).T        # [dm, tok]
    # [tb, g, p, ic_in*512+t] = xT[g*512+ic_in*128+p, tb*512+t]
    xt_host = np.ascontiguousarray(
        xT.reshape(8, 4, 128, 8, 512).transpose(3, 0, 2, 1, 4)
        .reshape(8, 8, 128, 2048)).astype(bf)
    cosT = np.ascontiguousarray(np.asarray(freqs_cos, np.float32).T).astype(bf)
    sinT = np.ascontiguousarray(np.asarray(freqs_sin, np.float32).T).astype(bf)
    mask = np.asarray(mask, np.float32)

    in_maps = []
    for c in range(N_CORES):
        m = {
            "xT": xt_host,
            "wq": tile_w(_deinterleave(
                np.asarray(wq[:, c * QW:(c + 1) * QW], np.float32))),
            "wk": tile_w(_deinterleave(
                np.asarray(wk[:, c * HD:(c + 1) * HD], np.float32))),
            "wv": tile_w(np.asarray(wv[:, c * HD:(c + 1) * HD], np.float32)),
            "wo": tile_w(np.asarray(wo[:, c * QW:(c + 1) * QW], np.float32)),
            "cosT": cosT,
            "sinT": sinT,
        }
        if causal:
            md = np.empty((4, 4, 128, 512), np.float32)
            for qb in range(4):
                blk = mask[qb * 512:(qb + 1) * 512,
                           qb * 512:(qb + 1) * 512]          # [q, k]
                md[qb] = blk.T.reshape(4, 128, 512)          # [j, k128, q512]
            m["maskd"] = np.ascontiguousarray(
                md.transpose(2, 0, 1, 3)).astype(bf)         # [p, qb, j, t]
        else:
            m["maskT"] = np.ascontiguousarray(mask.T).astype(bf)
        in_maps.append(m)
    return in_maps


def _is_causal(mask):
    mask = np.asarray(mask, np.float32)
    expect = np.where(np.tril(np.ones((S, S), bool)), 0.0, NEG).astype(np.float32)
    return np.array_equal(mask, expect)


def kernel(x, wq, wk, wv, wo, cache_k, cache_v, freqs_cos, freqs_sin, mask,
           start_pos):
    assert int(start_pos) == 0, "kernel hardcodes start_pos=0 prefill"
    assert tuple(np.shape(x)) == (B, S, D)
    causal = _is_causal(mask)
    nc = _get_nc(causal)
    in_maps = make_in_maps(x, wq, wk, wv, wo, freqs_cos, freqs_sin, mask,
                           causal)
    res = bass_utils.run_bass_kernel_spmd(
        nc, in_maps, core_ids=list(range(N_CORES)))
    out = np.empty((TOK, D), np.float32)
    for c in range(N_CORES):
        out[:, c * QW:(c + 1) * QW] = res.results[c]["out"]
    return out.reshape(B, S, D)


# revision 20
# speedup vs baseline: 1.0270x; 1.0038x over previous
"""Trainium2 Bass kernel: Llama-style attention prefill (B=2, S=2048, D=4096,
32 Q heads / 8 KV heads, head_dim 128, RoPE, additive mask), tensor-parallel
over heads across 8 NeuronCores.

Sharding (per core c):
  - Q heads 4c..4c+3 (wq columns c*512:(c+1)*512), KV head c (wk/wv columns
    c*128:(c+1)*128), wo column-shard wo[:, c*512:(c+1)*512].
  - Each core computes QKV projections + RoPE + attention for its heads,
    AllGathers the (transposed) attention outputs over all 8 cores (one AG
    per 512-token chunk, overlapped with compute), then computes a
    512-column slice of the output projection.
  - Host concatenates the 8 column slices -> full output.

Device-side layouts (all "T" = feature-on-partitions):
  - xT [4096 dm, 4096 tok] (tok = b*2048 + s), host-provided, bf16.
  - wq/wk columns are de-interleaved per head on the host: new col order
    [re pairs 0..63 | im pairs 0..63], so RoPE acts on partition halves.
    Scores are invariant (same permutation on Q and K); V/wo untouched.
  - Scores computed transposed: ST[k, t] = K @ Q^T; softmax over the
    partition axis k: exp (no max subtraction; |scores|*scale stays small
    so f32 exp is safe), then PV matmul with a ones-column appended to V
    producing both O[t, d] and the denominator L[t].
  - O^T staging for the AllGather is done ON-CHIP: each normalized o_n
    [128 t, 128 d] tile is transposed by the PE (is_transpose matmul with
    a bf16 identity -> bf16 PSUM tile), collected per head as [128 d,
    512 t], and DMA'd PSUM -> ag_in DRAM directly. No DRAM roundtrip and
    no xbar-mode DMA transposes (which would serialize against all
    previously-emitted collectives under the conservative xbar rule).
  - Scores/exp exploit causality at 128-row granularity: for the
    diagonal 512-block, key sub-block j only computes query columns
    >= j*128 (the PV accumulation skips the complementary tiles).
  - Emission: per qb, project the two 512-token blocks then run both
    chunks' attention + AllGather; final phase runs the qb=3 attention,
    triggers AG3, then all eight out-projection chunks (groups 0-2's
    AGs completed long ago; AG3 flies under the first six chunks).
    AG staging DMAs and PSUM->SBUF copies ride the otherwise-idle
    gpsimd queue so they are never stuck behind og/out traffic and
    each AG trigger fires as soon as its inputs land.
"""
import numpy as np
import ml_dtypes

from concourse import bass, bacc, tile, mybir, bass_utils
from concourse.masks import make_identity

F32 = mybir.dt.float32
BF16 = mybir.dt.bfloat16
Alu = mybir.AluOpType
Act = mybir.ActivationFunctionType

N_CORES = 8
B, S, D = 2, 2048, 4096
TOK = B * S                      # 4096 flattened tokens
HD = 128                         # head dim
HQ = 4                           # q heads per core
QW = HQ * HD                     # 512, per-core q width
SCALE = 1.0 / float(np.sqrt(HD))
NEG = -1e9

_BUILD_CACHE = {}


def _build(causal: bool):
    nc = bacc.Bacc("TRN2", target_bir_lowering=False, debug=False,
                   num_devices=N_CORES)
    # all inputs are pre-tiled on the host so every DMA is one contiguous
    # segment per partition (cheap descriptor generation)
    xT_d = nc.dram_tensor("xT", [8, 8, 128, 2048], BF16, kind="ExternalInput")
    wq_d = nc.dram_tensor("wq", [128, 32, QW], BF16, kind="ExternalInput")
    wk_d = nc.dram_tensor("wk", [128, 32, HD], BF16, kind="ExternalInput")
    wv_d = nc.dram_tensor("wv", [128, 32, HD], BF16, kind="ExternalInput")
    wo_d = nc.dram_tensor("wo", [128, 32, QW], BF16, kind="ExternalInput")
    cosT_d = nc.dram_tensor("cosT", [64, S], BF16, kind="ExternalInput")
    sinT_d = nc.dram_tensor("sinT", [64, S], BF16, kind="ExternalInput")
    if causal:
        # diagonal 512x512 blocks of mask^T, tiled [qb, j, 128, 512]
        maskd_d = nc.dram_tensor("maskd", [128, 4, 4, 512], BF16,
                                 kind="ExternalInput")
    else:
        maskT_d = nc.dram_tensor("maskT", [S, S], BF16, kind="ExternalInput")
    out_d = nc.dram_tensor("out", [TOK, QW], F32, kind="ExternalOutput")

    with tile.TileContext(nc) as tc:
        with (
            tc.tile_pool(name="res", bufs=1) as res,            # residents
            tc.tile_pool(name="qtp", bufs=4 if causal else 8) as qtp,
            tc.tile_pool(name="ph2", bufs=2) as ph2,
            tc.tile_pool(name="epool", bufs=18) as epool,
            tc.tile_pool(name="mpool", bufs=9) as mpool,
            tc.tile_pool(name="onp", bufs=4) as onp,
            tc.tile_pool(name="agd", bufs=8, space="DRAM") as agd,
            tc.tile_pool(name="psum", bufs=2, space="PSUM") as psum,
        ):
            # ---- resident loads (spread across queues) ----
            wk_sb = res.tile([128, 32, HD], BF16, name="wk_sb")
            # small first chunk so the very first matmul's LDWEIGHTS isn't
            # gated on the whole 1MB tile (plus cold DMA-queue ramp)
            nc.gpsimd.dma_start(wk_sb[:, 0:4], wk_d[:, 0:4])
            nc.gpsimd.dma_start(wk_sb[:, 4:32], wk_d[:, 4:32])
            wv_sb = res.tile([128, 32, HD], BF16, name="wv_sb")
            nc.gpsimd.dma_start(wv_sb[:], wv_d[:])
            cos_sb = res.tile([64, S], BF16, name="cos_sb")
            nc.gpsimd.dma_start(cos_sb[:], cosT_d[:])
            sin_sb = res.tile([64, S], BF16, name="sin_sb")
            nc.gpsimd.dma_start(sin_sb[:], sinT_d[:])
            if causal:
                mask_sb = res.tile([128, 4, 4, 512], BF16, name="mask_sb")
                nc.gpsimd.dma_start(mask_sb[:], maskd_d[:])
            # identity for the PE transposes; created after the resident
            # DMAs so the gpsimd ops don't delay the first K-proj weights
            ident = res.tile([128, 128], BF16, name="ident")
            make_identity(nc, ident[:])

            # per-token-block K/V residents (block granularity keeps the
            # projection->attention dependency tracking per-block)
            kt_t = [res.tile([128, 512], BF16, name=f"kt{tb}")
                    for tb in range(8)]
            v_t = [res.tile([128, 4, 130], BF16, name=f"v{tb}")
                   for tb in range(8)]
            for tb in range(8):
                nc.vector.memset(v_t[tb][:, :, 128:129], 1.0)

            # ---------------- projections + RoPE for one 512-token block --
            def rope_store(ps, out_re, out_im, cos_sl, sin_sl, rp):
                t1 = rp.tile([64, 512], F32, name="t1", tag="t1", bufs=1)
                t2 = rp.tile([64, 512], F32, name="t2", tag="t2", bufs=1)
                nc.vector.tensor_mul(t1[:], ps[0:64, :], cos_sl)
                nc.vector.tensor_mul(t2[:], ps[64:128, :], sin_sl)
                nc.vector.tensor_sub(out_re, t1[:], t2[:])
                nc.vector.tensor_mul(t1[:], ps[0:64, :], sin_sl)
                nc.vector.tensor_mul(t2[:], ps[64:128, :], cos_sl)
                nc.vector.tensor_add(out_im, t1[:], t2[:])

            def proj_block(tb, wq_sb, xtp, ph1):
                xts = []
                for g in range(8):
                    xt = xtp.tile([128, 2048], BF16, name="xt", tag="xt")
                    nc.sync.dma_start(xt[:], xT_d[tb, g])
                    xts.append(xt)

                def xsl(ic):
                    return xts[ic // 4][:, (ic % 4) * 512:(ic % 4 + 1) * 512]

                pos = (tb % 4) * 512
                cos_sl = cos_sb[:, pos:pos + 512]
                sin_sl = sin_sb[:, pos:pos + 512]

                ps_k = psum.tile([128, 512], F32, name="ps_k", tag="ps")
                for ic in range(32):
                    nc.tensor.matmul(ps_k[:], wk_sb[:, ic, :], xsl(ic),
                                     start=(ic == 0), stop=(ic == 31))
                rope_store(ps_k, kt_t[tb][0:64, :], kt_t[tb][64:128, :],
                           cos_sl, sin_sl, ph1)

                # V in natural [token, d] layout, computed directly:
                # lhsT = xT chunk [i, t(128)], rhs = wv chunk [i, d]
                for j in range(4):
                    ps_v = psum.tile([128, 128], F32, name="ps_v", tag="ps")
                    for ic in range(32):
                        nc.tensor.matmul(
                            ps_v[:],
                            xsl(ic)[:, j * 128:(j + 1) * 128],
                            wv_sb[:, ic, :],
                            start=(ic == 0), stop=(ic == 31))
                    nc.vector.tensor_copy(v_t[tb][:, j, 0:128], ps_v[:])

                qt = qtp.tile([128, HQ, 512], BF16, name="qt", tag="qt")
                for dq in range(HQ):
                    ps_q = psum.tile([128, 512], F32, name="ps_q", tag="ps")
                    for ic in range(32):
                        nc.tensor.matmul(
                            ps_q[:],
                            wq_sb[:, ic, dq * HD:(dq + 1) * HD],
                            xsl(ic),
                            start=(ic == 0), stop=(ic == 31))
                    rope_store(ps_q, qt[0:64, dq, :], qt[64:128, dq, :],
                               cos_sl, sin_sl, ph1)
                return qt

            # three AllGathers (A: qb0-1 chunks, B: qb2, C: qb3), sized so
            # AG-A/B trigger mid-kernel and AG-C hides under the out-
            # projections of A+B
            GRP_SLOTS = [2, 2, 2, 2]
            ag_in_g = [agd.tile([512, 512 * n], BF16, name=f"ag_in{g}")
                       for g, n in enumerate(GRP_SLOTS)]
            ag_out_g = [agd.tile([D, 512 * n], BF16, name=f"ag_out{g}",
                                 addr_space="Shared")
                        for g, n in enumerate(GRP_SLOTS)]

            def chunk_group(qb, b):
                # (group, slot) for chunk (qb, b)
                return qb, b

            # ---------------- attention + AG staging for one chunk --------
            def attn_chunk(qb, b, qt, mtiles, kcs, qoff):
                # qoff[kc]: first valid query column for block kc (causal
                # diagonal sub-blocks only attend q >= k, so the scores /
                # exp / PV for q < qoff[kc] are skipped entirely)
                g, slot = chunk_group(qb, b)
                for h in range(HQ):
                    e_ps = {}
                    for kc in kcs:
                        qo = qoff.get(kc, 0)
                        st_ps = psum.tile([128, 512], F32, name="st_ps",
                                          tag="st")
                        tbk = b * 4 + kc // 4
                        kof = (kc % 4) * 128
                        nc.tensor.matmul(
                            st_ps[:, qo:],
                            kt_t[tbk][:, kof:kof + 128],
                            qt[:, h, qo:],
                            start=True, stop=True)
                        e_t = epool.tile([128, 512], BF16, name="e_t",
                                         tag="e")
                        if mtiles.get(kc) is not None:
                            pre = ph2.tile([128, 512], F32, name="pre",
                                           tag="pre")
                            nc.vector.scalar_tensor_tensor(
                                pre[:, qo:], st_ps[:, qo:], SCALE,
                                mtiles[kc][:, qo:], Alu.mult, Alu.add)
                            nc.scalar.activation(e_t[:, qo:], pre[:, qo:],
                                                 Act.Exp)
                        else:
                            nc.scalar.activation(e_t[:, qo:], st_ps[:, qo:],
                                                 Act.Exp, scale=SCALE)
                        e_ps[kc] = e_t
                    # per-head transposed output collector [128 d, 512 t],
                    # bf16 PSUM (PE transpose writes lhsT dtype)
                    tr = psum.tile([128, 512], BF16, name="tr", tag="tr")
                    for ts in range(4):
                        tcs = [kc for kc in kcs
                               if qoff.get(kc, 0) <= ts * 128]
                        o_ps = psum.tile([128, 129], F32, name="o_ps", tag="o")
                        for i, kc in enumerate(tcs):
                            nc.tensor.matmul(
                                o_ps[:],
                                e_ps[kc][:, ts * 128:(ts + 1) * 128],
                                v_t[b * 4 + kc // 4][:, kc % 4, 0:129],
                                start=(i == 0),
                                stop=(i == len(tcs) - 1))
                        linv = onp.tile([128, 1], F32, name="linv", tag="linv")
                        nc.vector.reciprocal(linv[:], o_ps[:, 128:129])
                        o_n = onp.tile([128, 128], BF16, name="o_n", tag="o_n")
                        nc.vector.tensor_scalar(
                            o_n[:], o_ps[:, 0:128], linv[:], None, Alu.mult)
                        nc.tensor.transpose(
                            tr[:, ts * 128:(ts + 1) * 128], o_n[:], ident[:])
                    trs = onp.tile([128, 512], BF16, name="trs", tag="trs")
                    # copy on vector: the scalar engine is co-critical with
                    # the PE during attention (exp stream); gpsimd cannot
                    # read PSUM
                    nc.vector.tensor_copy(trs[:], tr[:])
                    # staging DMAs ride the gpsimd queue: it holds only the
                    # AG triggers, so staging is never queued behind og /
                    # out traffic and each trigger fires right after its
                    # inputs land
                    nc.gpsimd.dma_start(
                        ag_in_g[g][h * 128:(h + 1) * 128,
                                   slot * 512:(slot + 1) * 512],
                        trs[:])

            def ag_group(g):
                nc.gpsimd.collective_compute(
                    "AllGather", Alu.bypass,
                    replica_groups=[list(range(N_CORES))],
                    ins=[ag_in_g[g][:]], outs=[ag_out_g[g][:]])

            # ---------------- emission ------------------------------------
            def make_mtiles(qb):
                if causal:
                    kcs = list(range(4 * qb + 4))
                    mtiles = {4 * qb + j: mask_sb[:, qb, j, :]
                              for j in range(4)}
                    qoff = {4 * qb + j: j * 128 for j in range(4)}
                    return mtiles, kcs, qoff
                else:
                    kcs = list(range(16))
                    mtiles = {}
                    for kc in range(0, 16, 2):
                        mt = mpool.tile([128, 2, 512], BF16, name="mt",
                                        tag="mt")
                        nc.sync.dma_start(
                            mt[:],
                            maskT_d[kc * 128:(kc + 2) * 128,
                                    qb * 512:(qb + 1) * 512]
                            .rearrange("(two p) t -> p two t", p=128))
                        mtiles[kc] = mt[:, 0, :]
                        mtiles[kc + 1] = mt[:, 1, :]
                return mtiles, kcs, {}

            with (
                tc.tile_pool(name="wqp", bufs=1) as wqp,
                tc.tile_pool(name="xtp", bufs=16 if causal else 8) as xtp,
                tc.tile_pool(name="ph1", bufs=2) as ph1,
            ):
                wq_sb = wqp.tile([128, 32, QW], BF16, name="wq_sb")
                nc.scalar.dma_start(wq_sb[:], wq_d[:])
                # tiny dummy AllGather emitted after the weight loads (the
                # collective trigger blocks the gpsimd queue): aligns the
                # cores' start skew while the first projection blocks keep
                # the PE busy, so AG0 isn't delayed by launch offset
                dummy_in = agd.tile([64, 4], BF16, name="dummy_in")
                nc.gpsimd.dma_start(dummy_in[:], cosT_d[0:64, 0:4])
                dummy_out = agd.tile([512, 4], BF16, name="dummy_out",
                                     addr_space="Shared")
                nc.gpsimd.collective_compute(
                    "AllGather", Alu.bypass,
                    replica_groups=[list(range(N_CORES))],
                    ins=[dummy_in[:]], outs=[dummy_out[:]])
                if causal:
                    # causal: chunk (qb, b) needs only blocks <= qb, so
                    # projections and attention interleave per qb
                    for qb in range(4):
                        qts = [proj_block(b * 4 + qb, wq_sb, xtp, ph1)
                               for b in range(B)]
                        mtiles, kcs, qoff = make_mtiles(qb)
                        if qb < 3:
                            for b in range(B):
                                attn_chunk(qb, b, qts[b], mtiles, kcs, qoff)
                            ag_group(qb)
                        else:
                            qts3, mtiles3, kcs3, qoff3 = qts, mtiles, kcs, \
                                qoff
                else:
                    # general mask: every chunk may attend to every block,
                    # so all projections must complete first
                    allq = [[None, None] for _ in range(4)]
                    for qb in range(4):
                        for b in range(B):
                            allq[qb][b] = proj_block(b * 4 + qb, wq_sb, xtp,
                                                     ph1)
                    for qb in range(3):
                        mtiles, kcs, qoff = make_mtiles(qb)
                        for b in range(B):
                            attn_chunk(qb, b, allq[qb][b], mtiles, kcs, qoff)
                        ag_group(qb)
                    qts3 = allq[3]
                    mtiles3, kcs3, qoff3 = make_mtiles(3)

            # final phase: wo reuses wq's SBUF lifetime. Order: qb=3
            # attention chunks (staging lands in ag_in[3]), then the six
            # out-projection chunks of groups 0-2 (their AGs completed long
            # before), with AG3 - whose gpsimd trigger has nothing queued
            # ahead of it but the wo load - flying underneath, then the two
            # group-3 out-projections.
            with (
                tc.tile_pool(name="wop", bufs=1) as wop,
                tc.tile_pool(name="ogp", bufs=4) as ogp,
            ):
                wo_sb = wop.tile([128, 32, QW], BF16, name="wo_sb")
                nc.gpsimd.dma_start(wo_sb[:], wo_d[:])

                def og_load(ag_out, slot, tc2):
                    # quarter-chunk og tile; 4-deep ring gives ~3 loads of
                    # prefetch so the attn3 -> out-proj boundary and og/AG
                    # HBM contention never stall the PE
                    og_sb = ogp.tile([128, 32, 128], BF16, name="og_sb",
                                     tag="og")
                    nc.sync.dma_start(
                        og_sb[:],
                        ag_out[:, slot * 512 + tc2 * 128:
                               slot * 512 + (tc2 + 1) * 128]
                        .rearrange("(hc p) t -> p hc t", p=128))
                    return og_sb

                def out_proj_q(og_sb, t0, tc2):
                    op_ps = psum.tile([128, 512], F32, name="op_ps",
                                      tag="ps")
                    for hc in range(32):
                        nc.tensor.matmul(
                            op_ps[:],
                            og_sb[:, hc, :],
                            wo_sb[:, hc, :],
                            start=(hc == 0), stop=(hc == 31))
                    oo = ph2.tile([128, 512], F32, name="oo", tag="oo")
                    nc.vector.tensor_copy(oo[:], op_ps[:])
                    nc.scalar.dma_start(
                        out_d[t0 + tc2 * 128: t0 + (tc2 + 1) * 128, :],
                        oo[:])

                for b3 in range(B):
                    attn_chunk(3, b3, qts3[b3], mtiles3, kcs3, qoff3)
                # out-projections for groups 0-2 are emitted BEFORE the
                # final collective so their og loads can never order
                # behind it; AG3's trigger sits on the clean gpsimd queue
                # right after the attn3 staging DMAs and flies under them
                quarters = []
                for qb in range(4):
                    for b in range(B):
                        g, slot = chunk_group(qb, b)
                        for tc2 in range(4):
                            quarters.append((g, slot, b * S + qb * 512, tc2))
                for g, slot, t0, tc2 in quarters[:24]:
                    out_proj_q(og_load(ag_out_g[g], slot, tc2), t0, tc2)
                ag_group(3)
                for g, slot, t0, tc2 in quarters[24:]:
                    out_proj_q(og_load(ag_out_g[g], slot, tc2), t0, tc2)

    nc.compile()
    return nc


def _get_nc(causal: bool):
    if causal not in _BUILD_CACHE:
        _BUILD_CACHE[causal] = _build(causal)
    return _BUILD_CACHE[causal]


_DEINT = np.concatenate([np.arange(0, HD, 2), np.arange(1, HD, 2)])


def _deinterleave(w):
    """Permute per-head columns [0,2,..,126,1,3,..,127] (re block | im block)."""
    out = w.copy()
    nh = w.shape[1] // HD
    for h in range(nh):
        out[:, h * HD:(h + 1) * HD] = w[:, h * HD + _DEINT]
    return out


def make_in_maps(x, wq, wk, wv, wo, freqs_cos, freqs_sin, mask, causal):
    bf = ml_dtypes.bfloat16

    def tile_w(w):
        # [4096, W] -> [128, 32, W]: out[p, ic, d] = w[ic*128+p, d]
        return np.ascontiguousarray(
            w.reshape(32, 128, -1).transpose(1, 0, 2)).astype(bf)

    xT = np.asarray(x, np.float32).reshape(TOK, D).T        # [dm, tok]
    # [tb, g, p, ic_in*512+t] = xT[g*512+ic_in*128+p, tb*512+t]
    xt_host = np.ascontiguousarray(
        xT.reshape(8, 4, 128, 8, 512).transpose(3, 0, 2, 1, 4)
        .reshape(8, 8, 128, 2048)).astype(bf)
    cosT = np.ascontiguousarray(np.asarray(freqs_cos, np.float32).T).astype(bf)
    sinT = np.ascontiguousarray(np.asarray(freqs_sin, np.float32).T).astype(bf)
    mask = np.asarray(mask, np.float32)

    in_maps = []
    for c in range(N_CORES):
        m = {
            "xT": xt_host,
            "wq": tile_w(_deinterleave(
                np.asarray(wq[:, c * QW:(c + 1) * QW], np.float32))),
            "wk": tile_w(_deinterleave(
                np.asarray(wk[:, c * HD:(c + 1) * HD], np.float32))),
            "wv": tile_w(np.asarray(wv[:, c * HD:(c + 1) * HD], np.float32)),
            "wo": tile_w(np.asarray(wo[:, c * QW:(c + 1) * QW], np.float32)),
            "cosT": cosT,
            "sinT": sinT,
        }
        if causal:
            md = np.empty((4, 4, 128, 512), np.float32)
            for qb in range(4):
                blk = mask[qb * 512:(qb + 1) * 512,
                           qb * 512:(qb + 1) * 512]          # [q, k]
                md[qb] = blk.T.reshape(4, 128, 512)          # [j, k128, q512]
            m["maskd"] = np.ascontiguousarray(
                md.transpose(2, 0, 1, 3)).astype(bf)         # [p, qb, j, t]
        else:
            m["maskT"] = np.ascontiguousarray(mask.T).astype(bf)
        in_maps.append(m)
    return in_maps


def _is_causal(mask):
    mask = np.asarray(mask, np.float32)
    expect = np.where(np.tril(np.ones((S, S), bool)), 0.0, NEG).astype(np.float32)
    return np.array_equal(mask, expect)


def kernel(x, wq, wk, wv, wo, cache_k, cache_v, freqs_cos, freqs_sin, mask,
           start_pos):
    assert int(start_pos) == 0, "kernel hardcodes start_pos=0 prefill"
    assert tuple(np.shape(x)) == (B, S, D)
    causal = _is_causal(mask)
    nc = _get_nc(causal)
    in_maps = make_in_maps(x, wq, wk, wv, wo, freqs_cos, freqs_sin, mask,
                           causal)
    res = bass_utils.run_bass_kernel_spmd(
        nc, in_maps, core_ids=list(range(N_CORES)))
    out = np.empty((TOK, D), np.float32)
    for c in range(N_CORES):
        out[:, c * QW:(c + 1) * QW] = res.results[c]["out"]
    return out.reshape(B, S, D)
